# revision 10
# baseline (speedup 1.0000x reference)
"""Trainium2 Bass kernel for nn_AEGConv2d (8 NeuronCores, SPMD).

Problem: out = sigmoid(aeg(x, weight)) * (conv2d(x, conv_w) + conv_b)
  x: (4, 32, 64, 64) f32, weight/conv_w: (64, 32, 3, 3), conv_b: (64,)
  stride=1, padding=1.

The AEG recurrence  res <- where(mask_k, (res+x_k)*y_k, (res+y_k)*x_k)
is affine in res with b_k = x_k*y_k in both branches and multiplier
a_k = y_k (mask true) or x_k (mask false).  mask(k, i, j) = (i+j+k)%2==0
is a checkerboard, so for a pixel of parity s=(i+j)%2 the unrolled sum
    res = sum_k x_k*y_k * prod_{j>k} a_j
factors into a per-pixel product A_k = x_k * prod_{j>k, j%2!=s} x_j and a
per-(cout,cin) product B_k = y_k * prod_{j>k, j%2==s} y_j, making the whole
AEG conv a 288-deep matmul per parity class -- same shape as the dense conv.

Sharding: 8 cores = 4 images x 2 row-halves (rows 0:32 / 32:64). Each core
computes all 64 Cout for its half image. No collectives; host gathers.

Per-core layout trick: the padded input slab (32cin, 34rows, 66cols) is
DMA'd into SBUF as 3 row-shifted replicas (partition p = ki*32+cin holds the
slab shifted by ki rows), so all 9 conv taps are pure strided views and the
dense-conv im2col needs no on-chip copies (K-chunks = kernel columns).
"""

import numpy as np

import concourse.bacc as bacc
import concourse.bass as bass
import concourse.mybir as mybir
import concourse.tile as tile
from concourse.bass_utils import run_bass_kernel_spmd

F32 = mybir.dt.float32
F32R = mybir.dt.float32r

N, CIN, H, W = 4, 32, 64, 64
COUT, KK = 64, 3
PAD = 1
OH, OW = 32, 64          # per-core output rows x cols
ROWS, COLS = 34, 66      # per-core padded slab
PITCH = ROWS * COLS      # 2244
FREE3 = 2112             # X3 usable free size per replica
N_CORES = 8

# suffix level needed by A_k at parity s (0 => raw tap, folded into split matmul)
SIGMA = {
    0: {0: 4, 1: 3, 2: 3, 3: 2, 4: 2, 5: 1, 6: 1, 7: 0, 8: 0},
    1: {0: 4, 1: 4, 2: 3, 3: 3, 4: 2, 5: 2, 6: 1, 7: 1, 8: 0},
}
CHAIN_TAPS = {0: [7, 5, 3, 1], 1: [8, 6, 4, 2]}
# kj -> raw tap handled by a K=32 matmul on the raw X3 view
RAW = {0: {1: 7, 2: 8}, 1: {2: 8}}

USE_F32R = True
MMDT = F32R  # dtype of all matmul-feeding tiles

_last_results = None  # stash for test.py (exec_time_ns etc.)


def _cast(ap):
    return ap


def _fview(base_ap, off, dims):
    """View with the same partition dim as base_ap but custom free dims."""
    return bass.AP(
        tensor=base_ap.tensor,
        offset=base_ap.offset + off,
        ap=[base_ap.ap[0]] + dims,
    )


def build_nc():
    nc = bacc.Bacc(None, target_bir_lowering=False)
    xslab = nc.declare_dram_parameter("xslab", [CIN * PITCH], F32, isOutput=False)
    wc_d = nc.declare_dram_parameter("wc", [96, 3, COUT], F32, isOutput=False)
    b0_d = nc.declare_dram_parameter("b0", [96, 3, COUT], F32, isOutput=False)
    b1_d = nc.declare_dram_parameter("b1", [96, 3, COUT], F32, isOutput=False)
    bias_d = nc.declare_dram_parameter("bias", [COUT, 1], F32, isOutput=False)
    out_d = nc.declare_dram_parameter("out", [COUT, OH * OW], F32, isOutput=True)

    with tile.TileContext(nc) as tc:
        with (
            tc.tile_pool(name="big", bufs=1) as big,
            tc.tile_pool(name="sig", bufs=4) as sigp,
            tc.tile_pool(name="psum", bufs=4, space="PSUM") as pp,
        ):
            # --- load inputs ---
            # X3: 3 row-shifted replicas (partition p = ki*32+cin) -> all 9
            # taps are strided views; 96-partition K-chunks for the matmuls.
            X3 = big.tile([96, FREE3], MMDT)
            xsb = xslab[:]
            dmae = [nc.sync, nc.scalar]
            CCH = FREE3 // 6  # 352-col chunks
            for i in range(6):
                dmae[i % 2].dma_start(
                    out=X3[:, CCH * i : CCH * (i + 1)],
                    in_=bass.AP(
                        tensor=xsb.tensor,
                        offset=CCH * i,
                        ap=[[COLS, 3], [PITCH, CIN], [1, CCH]],
                    ).bitcast(MMDT),
                )
            # Xg1/Xg2: base-partition-0 copies of the ki=1/2 replica groups.
            # DVE TensorTensor requires both SBUF inputs at the same base
            # partition, so all elementwise reads go through these.
            xg = {0: X3}
            for ki in (1, 2):
                t = big.tile([CIN, FREE3], MMDT, tag=f"xg{ki}")
                for i in range(2):
                    h = FREE3 // 2
                    dmae[(ki + i) % 2].dma_start(
                        out=t[:, h * i : h * (i + 1)],
                        in_=bass.AP(
                            tensor=xsb.tensor,
                            offset=66 * ki + h * i,
                            ap=[[PITCH, CIN], [1, h]],
                        ).bitcast(MMDT),
                    )
                xg[ki] = t
            wts = {}
            for name, dram in (("wc", wc_d), ("b0", b0_d), ("b1", b1_d)):
                t = big.tile([96, 3, COUT], MMDT, tag=name)
                nc.sync.dma_start(out=t[:, :, :], in_=dram[:, :, :].bitcast(MMDT))
                wts[name] = t
            bias_t = big.tile([COUT, 1], F32)
            nc.sync.dma_start(out=bias_t[:, :], in_=bias_d[:, :])

            out_sb = big.tile([COUT, OH * OW], F32)

            x3_all = X3[:, :]

            # Touch ops: absorb DMA-completion waits into DVE program order so
            # every TensorTensor needs at most one embedded sync wait
            # (S3S3D3_TT codegen supports only one).
            scratch = big.tile([1, 8], F32)
            touch_srcs = [X3[0:1, CCH * i : CCH * i + 1].bitcast(F32) for i in range(6)]
            touch_srcs += [xg[ki][0:1, (FREE3 // 2) * i : (FREE3 // 2) * i + 1].bitcast(F32)
                           for ki in (1, 2) for i in range(2)]
            touch_srcs.append(bias_t[0:1, 0:1])
            for tt in touch_srcs:
                nc.vector.tensor_copy(scratch[0:1, 0:1], tt)
            gscratch = big.tile([1, 8], F32, tag="gscratch")
            for tt in touch_srcs[:10]:
                nc.gpsimd.tensor_copy(gscratch[0:1, 0:1], tt)

            def tapview(k, s):
                """(32, 2,16,32) base-0 view of tap k at all parity-s pixels."""
                ki, kj = divmod(k, 3)
                src = xg[ki]
                base = src[0:32, :]
                return _fview(base, kj + s, [[67 - 2 * s, 2], [132, 16], [2, 32]])

            def x3grid(kj, s, t, ki=None):
                """(96 or 32, 16,32) view: kernel-column kj taps at grid t of parity s."""
                base = x3_all if ki is None else X3[32 * ki : 32 * ki + 32, :]
                off = kj + 66 * t + (s ^ t)
                return _fview(base, off, [[132, 16], [2, 32]])

            # --- chains + A tensors (elementwise products) ---
            A = {}
            for s in (0, 1):
                c = CHAIN_TAPS[s]
                chain = {1: tapview(c[0], s)}
                ceng = nc.vector if s == 0 else nc.gpsimd
                for lvl in (2, 3, 4):
                    ct = big.tile([32, 2, 16, 32], MMDT, tag=f"ch{s}{lvl}")
                    ceng.tensor_mul(ct[:, :, :, :], tapview(c[lvl - 1], s), chain[lvl - 1])
                    chain[lvl] = ct[:, :, :, :]
                for kj in range(3):
                    at = big.tile([96, 2, 16, 32], MMDT, tag=f"A{s}{kj}")
                    A[(s, kj)] = at
                    for ki in range(3):
                        k = ki * 3 + kj
                        lvl = SIGMA[s][k]
                        if lvl == 0:
                            continue  # raw tap: direct view matmul
                        aeng = nc.gpsimd if (s == 1 and kj == 2) else nc.vector
                        aeng.tensor_mul(
                            at[32 * ki : 32 * ki + 32, :, :, :],
                            tapview(k, s),
                            chain[lvl],
                        )

            # --- matmuls + epilogue per (parity, grid) ---
            bsn = {0: "b0", 1: "b1"}
            for s in (0, 1):
                for t in (0, 1):
                    ps_conv = pp.tile([64, 16, 32], F32, tag="ps_conv")
                    ps_aeg = pp.tile([64, 16, 32], F32, tag="ps_aeg")
                    # dense conv
                    for kj in range(3):
                        nc.tensor.matmul(
                            ps_conv[:, :, :],
                            _cast(wts["wc"][:, kj, :]),
                            _cast(x3grid(kj, s, t)),
                            start=(kj == 0),
                            stop=(kj == 2),
                        )
                    # aeg -> psum rows 64:128
                    bt = wts[bsn[s]]
                    mms = []
                    for kj in range(3):
                        if kj in RAW[s]:
                            mms.append((bt[0:64, kj, :], A[(s, kj)][0:64, t, :, :]))
                            mms.append((bt[64:96, kj, :], x3grid(kj, s, t, ki=2)))
                        else:
                            mms.append((bt[:, kj, :], A[(s, kj)][:, t, :, :]))
                    for i, (lh, rh) in enumerate(mms):
                        nc.tensor.matmul(
                            ps_aeg[:, :, :],
                            _cast(lh),
                            _cast(rh),
                            start=(i == 0),
                            stop=(i == len(mms) - 1),
                        )
                    sig = sigp.tile([64, 16, 32], F32)
                    nc.scalar.activation(
                        sig[:, :, :], ps_aeg[:, :, :],
                        mybir.ActivationFunctionType.Sigmoid,
                    )
                    # absorb the ACT-completion wait so the STT below only
                    # waits on PSUM (one embedded sync wait max)
                    nc.vector.tensor_copy(scratch[0:1, 0:1], sig[0:1, 0:1, 0:1])
                    # out = (conv + bias) * sigmoid(aeg), scattered to parity pixels
                    ov = _fview(out_sb[:, :], 64 * t + (s ^ t), [[128, 16], [2, 32]])
                    nc.vector.scalar_tensor_tensor(
                        out=ov,
                        in0=ps_conv[:, :, :],
                        scalar=bias_t[:, 0:1],
                        in1=sig[:, :, :],
                        op0=mybir.AluOpType.add,
                        op1=mybir.AluOpType.mult,
                    )

            for i in range(8):
                dmae[i % 2].dma_start(
                    out=out_d[8 * i : 8 * i + 8, :],
                    in_=out_sb[8 * i : 8 * i + 8, :],
                )
    nc.finalize()
    return nc


def _host_prep(x, weight, conv_w, conv_b):
    """Shard + pack per-core inputs."""
    xp = np.pad(np.ascontiguousarray(x, np.float32),
                ((0, 0), (0, 0), (PAD, PAD), (PAD, PAD)))
    kflat = weight.reshape(COUT, CIN, 9).transpose(2, 0, 1)  # (9, cout, cin)
    B = np.zeros((2, 9, COUT, CIN), np.float32)
    for s in (0, 1):
        suf = np.ones((COUT, CIN), np.float32)
        for k in range(8, -1, -1):
            B[s, k] = kflat[k] * suf
            if k % 2 == s:
                suf = suf * kflat[k]
    # lhsT packs: (96, 3, 64): [ki*32+cin, kj, cout]
    wc_p = np.ascontiguousarray(
        conv_w.transpose(2, 1, 3, 0).reshape(96, 3, COUT), np.float32
    )  # conv_w (cout,cin,ki,kj) -> (ki, cin, kj, cout)
    b_p = np.zeros((2, 96, 3, COUT), np.float32)
    for s in (0, 1):
        for kj in range(3):
            for ki in range(3):
                b_p[s, 32 * ki : 32 * ki + 32, kj] = B[s, ki * 3 + kj].T  # (cin, cout)
    bias_p = np.ascontiguousarray(conv_b.reshape(COUT, 1), np.float32)

    in_maps = []
    for core in range(N_CORES):
        n, h = divmod(core, 2)
        slab = np.ascontiguousarray(xp[n, :, 32 * h : 32 * h + ROWS, :], np.float32)
        in_maps.append({
            "xslab": slab.reshape(-1),
            "wc": wc_p,
            "b0": b_p[0],
            "b1": b_p[1],
            "bias": bias_p,
        })
    return in_maps


_nc_cache = None


def kernel(x, weight, conv_w, conv_b, trace=False):
    global _nc_cache, _last_results
    x = np.asarray(x, np.float32)
    weight = np.asarray(weight, np.float32)
    conv_w = np.asarray(conv_w, np.float32)
    conv_b = np.asarray(conv_b, np.float32)

    if _nc_cache is None:
        _nc_cache = build_nc()
    nc = _nc_cache
    in_maps = _host_prep(x, weight, conv_w, conv_b)
    res = run_bass_kernel_spmd(nc, in_maps, core_ids=list(range(N_CORES)), trace=trace)
    _last_results = res

    out = np.empty((N, COUT, H, W), np.float32)
    for core in range(N_CORES):
        n, h = divmod(core, 2)
        out[n, :, 32 * h : 32 * h + 32, :] = res.results[core]["out"].reshape(COUT, OH, OW)
    return out


# revision 12
# speedup vs baseline: 1.3671x; 1.3671x over previous
"""Trainium2 Bass kernel for nn_AEGConv2d (8 NeuronCores, SPMD).

Problem: out = sigmoid(aeg(x, weight)) * (conv2d(x, conv_w) + conv_b)
  x: (4, 32, 64, 64) f32, weight/conv_w: (64, 32, 3, 3), conv_b: (64,)
  stride=1, padding=1.

The AEG recurrence  res <- where(mask_k, (res+x_k)*y_k, (res+y_k)*x_k)
is affine in res with b_k = x_k*y_k in both branches and multiplier
a_k = y_k (mask true) or x_k (mask false).  mask(k, i, j) = (i+j+k)%2==0
is a checkerboard, so for a pixel of parity s=(i+j)%2 the unrolled sum
    res = sum_k x_k*y_k * prod_{j>k} a_j
factors into a per-pixel product A_k = x_k * prod_{j>k, j%2!=s} x_j and a
per-(cout,cin) product B_k = y_k * prod_{j>k, j%2==s} y_j, making the whole
AEG conv a 288-deep matmul per parity class -- same shape as the dense conv.

Sharding: 8 cores = 4 images x 2 row-halves (rows 0:32 / 32:64). Each core
computes all 64 Cout for its half image. No collectives; host gathers.

Per-core layout trick: the padded input slab (32cin, 34rows, 66cols) is
DMA'd into SBUF as 3 row-shifted replicas (partition p = ki*32+cin holds the
slab shifted by ki rows), so all 9 conv taps are pure strided views and the
dense-conv im2col needs no on-chip copies (K-chunks = kernel columns).
"""

import numpy as np

import concourse.bacc as bacc
import concourse.bass as bass
import concourse.mybir as mybir
import concourse.tile as tile
from concourse.bass_utils import run_bass_kernel_spmd

F32 = mybir.dt.float32
F32R = mybir.dt.float32r

N, CIN, H, W = 4, 32, 64, 64
COUT, KK = 64, 3
PAD = 1
OH, OW = 32, 64          # per-core output rows x cols
ROWS, COLS = 34, 66      # per-core padded slab
PITCH = ROWS * COLS      # 2244
FREE3 = 2112             # X3 usable free size per replica
N_CORES = 8

# suffix level needed by A_k at parity s (0 => raw tap, folded into split matmul)
SIGMA = {
    0: {0: 4, 1: 3, 2: 3, 3: 2, 4: 2, 5: 1, 6: 1, 7: 0, 8: 0},
    1: {0: 4, 1: 4, 2: 3, 3: 3, 4: 2, 5: 2, 6: 1, 7: 1, 8: 0},
}
CHAIN_TAPS = {0: [7, 5, 3, 1], 1: [8, 6, 4, 2]}
# kj -> raw tap handled by a K=32 matmul on the raw X3 view
RAW = {0: {1: 7, 2: 8}, 1: {2: 8}}

USE_F32R = True
MMDT = F32R  # dtype of all matmul-feeding tiles

_last_results = None  # stash for test.py (exec_time_ns etc.)


def _cast(ap):
    return ap


def _fview(base_ap, off, dims):
    """View with the same partition dim as base_ap but custom free dims."""
    return bass.AP(
        tensor=base_ap.tensor,
        offset=base_ap.offset + off,
        ap=[base_ap.ap[0]] + dims,
    )


def build_nc():
    nc = bacc.Bacc(None, target_bir_lowering=False)
    xslab = nc.declare_dram_parameter("xslab", [CIN * PITCH], F32, isOutput=False)
    wc_d = nc.declare_dram_parameter("wc", [CIN, 9, COUT], F32, isOutput=False)
    b0_d = nc.declare_dram_parameter("b0", [96, 3, COUT], F32, isOutput=False)
    b1_d = nc.declare_dram_parameter("b1", [96, 3, COUT], F32, isOutput=False)
    braw_d = nc.declare_dram_parameter("braw", [CIN, 3, COUT], F32, isOutput=False)
    bias_d = nc.declare_dram_parameter("bias", [COUT, 1], F32, isOutput=False)
    out_d = nc.declare_dram_parameter("out", [COUT, OH * OW], F32, isOutput=True)

    with tile.TileContext(nc) as tc:
        with (
            tc.tile_pool(name="big", bufs=1) as big,
            tc.tile_pool(name="sig", bufs=4) as sigp,
            tc.tile_pool(name="psum", bufs=4, space="PSUM") as pp,
        ):
            # --- load inputs (minimal bytes; slab loaded once) ---
            XS = big.tile([CIN, PITCH], MMDT)
            xsb = xslab[:]
            dmae = [nc.sync, nc.scalar]
            NCH = 4
            CCH = PITCH // NCH  # 561-col chunks
            for i in range(NCH):
                dmae[i % 2].dma_start(
                    out=XS[:, CCH * i : CCH * (i + 1)],
                    in_=bass.AP(
                        tensor=xsb.tensor,
                        offset=CCH * i,
                        ap=[[PITCH, CIN], [1, CCH]],
                    ).bitcast(MMDT),
                )
            wts = {}
            for name, dram, shape in (("wc", wc_d, [CIN, 9, COUT]),
                                      ("b0", b0_d, [96, 3, COUT]),
                                      ("b1", b1_d, [96, 3, COUT]),
                                      ("braw", braw_d, [CIN, 3, COUT])):
                t = big.tile(shape, MMDT, tag=name)
                ap = dram[:, :, :]
                dmae[len(wts) % 2].dma_start(out=t[:, :, :], in_=ap.bitcast(MMDT))
                wts[name] = t
            bias_t = big.tile([COUT, 1], F32)
            nc.sync.dma_start(out=bias_t[:, :], in_=bias_d[:, :])

            out_sb = big.tile([COUT, OH * OW], F32)

            # Touch ops: absorb DMA-completion waits into DVE program order so
            # every TensorTensor needs at most one embedded sync wait.
            scratch = big.tile([1, 8], F32)
            touch_srcs = [XS[0:1, CCH * i : CCH * i + 1].bitcast(F32) for i in range(NCH)]
            touch_srcs.append(bias_t[0:1, 0:1])
            for tt in touch_srcs:
                nc.vector.tensor_copy(scratch[0:1, 0:1], tt)

            def tapview(k, s):
                """(32, 2,16,32) base-0 view of tap k at all parity-s pixels."""
                ki, kj = divmod(k, 3)
                base = XS[:, :]
                return _fview(base, 66 * ki + kj + s,
                              [[67 - 2 * s, 2], [132, 16], [2, 32]])

            def tapgrid(k, s, t):
                """(32, 16,32) slab view of tap k at grid t of parity s."""
                ki, kj = divmod(k, 3)
                off = 66 * ki + kj + 66 * t + (s ^ t)
                return _fview(XS[:, :], off, [[132, 16], [2, 32]])

            # --- chains + A tensors (all on DVE; gpsimd interferes with DVE ports) ---
            A = {}
            for s in (0, 1):
                c = CHAIN_TAPS[s]
                chain = {1: tapview(c[0], s)}
                for lvl in (2, 3, 4):
                    ct = big.tile([32, 2, 16, 32], MMDT, tag=f"ch{s}{lvl}")
                    nc.vector.tensor_mul(ct[:, :, :, :], tapview(c[lvl - 1], s), chain[lvl - 1])
                    chain[lvl] = ct[:, :, :, :]
                for kj in range(3):
                    at = big.tile([96, 2, 16, 32], MMDT, tag=f"A{s}{kj}")
                    A[(s, kj)] = at
                    for ki in range(3):
                        k = ki * 3 + kj
                        lvl = SIGMA[s][k]
                        if lvl == 0:
                            continue  # raw tap: direct view matmul
                        nc.vector.tensor_mul(
                            at[32 * ki : 32 * ki + 32, :, :, :],
                            tapview(k, s),
                            chain[lvl],
                        )

            # --- matmuls + epilogue per (parity, grid) ---
            bsn = {0: "b0", 1: "b1"}
            for s in (0, 1):
                for t in (0, 1):
                    ps_conv = pp.tile([64, 16, 32], F32, tag="ps_conv")
                    ps_aeg = pp.tile([64, 16, 32], F32, tag="ps_aeg")
                    # dense conv: 9 K=32 chunks straight off the slab
                    for k in range(9):
                        nc.tensor.matmul(
                            ps_conv[:, :, :],
                            wts["wc"][:, k, :],
                            tapgrid(k, s, t),
                            start=(k == 0),
                            stop=(k == 8),
                        )
                    # aeg: kj-chunks on the materialized A tensors
                    bt = wts[bsn[s]]
                    mms = []
                    for kj in range(3):
                        if kj in RAW[s]:
                            slot = {(0, 1): 0, (0, 2): 1, (1, 2): 2}[(s, kj)]
                            mms.append((bt[0:64, kj, :], A[(s, kj)][0:64, t, :, :]))
                            mms.append((wts["braw"][:, slot, :], tapgrid(RAW[s][kj], s, t)))
                        else:
                            mms.append((bt[:, kj, :], A[(s, kj)][:, t, :, :]))
                    for i, (lh, rh) in enumerate(mms):
                        nc.tensor.matmul(
                            ps_aeg[:, :, :],
                            lh,
                            rh,
                            start=(i == 0),
                            stop=(i == len(mms) - 1),
                        )
                    sig = sigp.tile([64, 16, 32], F32)
                    nc.scalar.activation(
                        sig[:, :, :], ps_aeg[:, :, :],
                        mybir.ActivationFunctionType.Sigmoid,
                    )
                    # absorb the ACT-completion wait so the STT below only
                    # waits on PSUM (one embedded sync wait max)
                    nc.vector.tensor_copy(scratch[0:1, 0:1], sig[0:1, 0:1, 0:1])
                    # out = (conv + bias) * sigmoid(aeg), scattered to parity pixels
                    ov = _fview(out_sb[:, :], 64 * t + (s ^ t), [[128, 16], [2, 32]])
                    nc.vector.scalar_tensor_tensor(
                        out=ov,
                        in0=ps_conv[:, :, :],
                        scalar=bias_t[:, 0:1],
                        in1=sig[:, :, :],
                        op0=mybir.AluOpType.add,
                        op1=mybir.AluOpType.mult,
                    )

            for i in range(8):
                dmae[i % 2].dma_start(
                    out=out_d[8 * i : 8 * i + 8, :],
                    in_=out_sb[8 * i : 8 * i + 8, :],
                )
    nc.finalize()
    return nc


def _host_prep(x, weight, conv_w, conv_b):
    """Shard + pack per-core inputs."""
    xp = np.pad(np.ascontiguousarray(x, np.float32),
                ((0, 0), (0, 0), (PAD, PAD), (PAD, PAD)))
    kflat = weight.reshape(COUT, CIN, 9).transpose(2, 0, 1)  # (9, cout, cin)
    B = np.zeros((2, 9, COUT, CIN), np.float32)
    for s in (0, 1):
        suf = np.ones((COUT, CIN), np.float32)
        for k in range(8, -1, -1):
            B[s, k] = kflat[k] * suf
            if k % 2 == s:
                suf = suf * kflat[k]
    # conv lhsT: (32, 9, 64): [cin, k, cout]
    wc_p = np.ascontiguousarray(
        conv_w.reshape(COUT, CIN, 9).transpose(1, 2, 0), np.float32
    )
    b_p = np.zeros((2, 96, 3, COUT), np.float32)
    for s in (0, 1):
        for kj in range(3):
            for ki in range(3):
                b_p[s, 32 * ki : 32 * ki + 32, kj] = B[s, ki * 3 + kj].T  # (cin, cout)
    # raw-tap B columns, base-0 packed: slots (s=0,k=7), (s=0,k=8), (s=1,k=8)
    braw_p = np.zeros((CIN, 3, COUT), np.float32)
    braw_p[:, 0] = B[0, 7].T
    braw_p[:, 1] = B[0, 8].T
    braw_p[:, 2] = B[1, 8].T
    bias_p = np.ascontiguousarray(conv_b.reshape(COUT, 1), np.float32)

    in_maps = []
    for core in range(N_CORES):
        n, h = divmod(core, 2)
        slab = np.ascontiguousarray(xp[n, :, 32 * h : 32 * h + ROWS, :], np.float32)
        in_maps.append({
            "xslab": slab.reshape(-1),
            "wc": wc_p,
            "b0": b_p[0],
            "b1": b_p[1],
            "braw": braw_p,
            "bias": bias_p,
        })
    return in_maps


_nc_cache = None


def kernel(x, weight, conv_w, conv_b, trace=False):
    global _nc_cache, _last_results
    x = np.asarray(x, np.float32)
    weight = np.asarray(weight, np.float32)
    conv_w = np.asarray(conv_w, np.float32)
    conv_b = np.asarray(conv_b, np.float32)

    if _nc_cache is None:
        _nc_cache = build_nc()
    nc = _nc_cache
    in_maps = _host_prep(x, weight, conv_w, conv_b)
    res = run_bass_kernel_spmd(nc, in_maps, core_ids=list(range(N_CORES)), trace=trace)
    _last_results = res

    out = np.empty((N, COUT, H, W), np.float32)
    for core in range(N_CORES):
        n, h = divmod(core, 2)
        out[n, :, 32 * h : 32 * h + 32, :] = res.results[core]["out"].reshape(COUT, OH, OW)
    return out


# revision 13
# speedup vs baseline: 1.4804x; 1.0829x over previous
"""Trainium2 Bass kernel for nn_AEGConv2d (8 NeuronCores, SPMD).

Problem: out = sigmoid(aeg(x, weight)) * (conv2d(x, conv_w) + conv_b)
  x: (4, 32, 64, 64) f32, weight/conv_w: (64, 32, 3, 3), conv_b: (64,)
  stride=1, padding=1.

The AEG recurrence  res <- where(mask_k, (res+x_k)*y_k, (res+y_k)*x_k)
is affine in res with b_k = x_k*y_k in both branches and multiplier
a_k = y_k (mask true) or x_k (mask false).  mask(k, i, j) = (i+j+k)%2==0
is a checkerboard, so for a pixel of parity s=(i+j)%2 the unrolled sum
    res = sum_k x_k*y_k * prod_{j>k} a_j
factors into a per-pixel product A_k = x_k * prod_{j>k, j%2!=s} x_j and a
per-(cout,cin) product B_k = y_k * prod_{j>k, j%2==s} y_j, making the whole
AEG conv a 288-deep matmul per parity class -- same shape as the dense conv.

Sharding: 8 cores = 4 images x 2 row-halves (rows 0:32 / 32:64). Each core
computes all 64 Cout for its half image. No collectives; host gathers.

Layout: the host packs the padded input slab into bf16 checkerboard parity
planes plane_q[cin, r, ch] = xp[cin, r, 2*ch + (q+r)%2] (plus one-element-
shifted copies of each plane) so that every tap view -- both the per-parity
elementwise views and the matmul rhs grids -- is a unit-minor-stride,
4B-aligned AP of a single 32-partition SBUF tensor.  bf16 gives the DVE 2x
mode (720ns vs 1360ns per 1024-elem op) and the PE 1 cycle/row matmuls.
"""

import numpy as np
import ml_dtypes

import concourse.bacc as bacc
import concourse.bass as bass
import concourse.mybir as mybir
import concourse.tile as tile
from concourse.bass_utils import run_bass_kernel_spmd

F32 = mybir.dt.float32
BF16 = mybir.dt.bfloat16

N, CIN, H, W = 4, 32, 64, 64
COUT, KK = 64, 3
PAD = 1
OH, OW = 32, 64          # per-core output rows x cols
ROWS, COLS = 34, 66      # per-core padded slab
PLP = 34                 # plane row pitch (even for alignment)
PLSZ = PLP * ROWS        # 1156 elements per plane per cin
N_CORES = 8

# suffix level needed by A_k at parity s (0 => raw tap, direct matmul)
SIGMA = {
    0: {0: 4, 1: 3, 2: 3, 3: 2, 4: 2, 5: 1, 6: 1, 7: 0, 8: 0},
    1: {0: 4, 1: 4, 2: 3, 3: 3, 4: 2, 5: 2, 6: 1, 7: 1, 8: 0},
}
CHAIN_TAPS = {0: [7, 5, 3, 1], 1: [8, 6, 4, 2]}
# kj -> raw tap handled by a K=32 matmul on the raw plane view
RAW = {0: {1: 7, 2: 8}, 1: {2: 8}}

_last_results = None  # stash for test.py (exec_time_ns etc.)


def _fview(base_ap, off, dims):
    """View with the same partition dim as base_ap but custom free dims."""
    return bass.AP(
        tensor=base_ap.tensor,
        offset=base_ap.offset + off,
        ap=[base_ap.ap[0]] + dims,
    )


def _plane_off(k, s, t):
    """Aligned offset of tap k, parity s, grid t inside the XP tensor.

    XP free layout: [plane0 | plane1 | plane0shift | plane1shift], each PLSZ.
    Element (r', ch) of plane q is at q*PLSZ + r'*PLP + ch; the shifted copies
    hold plane[q][..., ch+1] so an odd offset o in plane q equals the even
    offset o-1 in plane q+2.
    """
    ki, kj = divmod(k, 3)
    q = (s + ki + kj) % 2
    m = ((s ^ t) + kj) // 2
    off = q * PLSZ + (t + ki) * PLP + m
    if off % 2 == 1:
        off = (2 + q) * PLSZ + (off - q * PLSZ) - 1
    return off


def build_nc():
    nc = bacc.Bacc(None, target_bir_lowering=False)
    xp_d = nc.declare_dram_parameter("xp", [CIN, 4 * PLSZ], BF16, isOutput=False)
    wc_d = nc.declare_dram_parameter("wc", [CIN, 9, COUT], BF16, isOutput=False)
    b0_d = nc.declare_dram_parameter("b0", [96, 3, COUT], BF16, isOutput=False)
    b1_d = nc.declare_dram_parameter("b1", [96, 3, COUT], BF16, isOutput=False)
    braw_d = nc.declare_dram_parameter("braw", [CIN, 3, COUT], BF16, isOutput=False)
    bias_d = nc.declare_dram_parameter("bias", [COUT, 1], F32, isOutput=False)
    out_d = nc.declare_dram_parameter("out", [COUT, OH * OW], BF16, isOutput=True)

    with tile.TileContext(nc) as tc:
        with (
            tc.tile_pool(name="big", bufs=1) as big,
            tc.tile_pool(name="sig", bufs=4) as sigp,
            tc.tile_pool(name="psum", bufs=4, space="PSUM") as pp,
        ):
            # --- load inputs: one DMA per plane copy (4 queues in parallel) ---
            XP = big.tile([CIN, 4 * PLSZ], BF16)
            xpb = xp_d[:, :]
            dmae = [nc.sync, nc.scalar]
            for i in range(4):
                dmae[i % 2].dma_start(
                    out=XP[:, PLSZ * i : PLSZ * (i + 1)],
                    in_=bass.AP(
                        tensor=xpb.tensor,
                        offset=PLSZ * i,
                        ap=[[4 * PLSZ, CIN], [1, PLSZ]],
                    ),
                )
            wts = {}
            for idx, (name, dram, shape) in enumerate((
                    ("wc", wc_d, [CIN, 9, COUT]),
                    ("b0", b0_d, [96, 3, COUT]),
                    ("b1", b1_d, [96, 3, COUT]),
                    ("braw", braw_d, [CIN, 3, COUT]))):
                t = big.tile(shape, BF16, tag=name)
                dmae[idx % 2].dma_start(out=t[:, :, :], in_=dram[:, :, :])
                wts[name] = t
            bias_t = big.tile([COUT, 1], F32)
            nc.sync.dma_start(out=bias_t[:, :], in_=bias_d[:, :])

            out_sb = big.tile([COUT, OH * OW], BF16)

            # Touch ops: absorb DMA-completion waits into DVE program order so
            # every TensorTensor needs at most one embedded sync wait.
            scratch = big.tile([1, 8], F32)
            touch_srcs = [XP[0:1, PLSZ * i : PLSZ * i + 1] for i in range(4)]
            touch_srcs.append(bias_t[0:1, 0:1])
            for tt in touch_srcs:
                nc.vector.tensor_copy(scratch[0:1, 0:1], tt)

            xp_all = XP[:, :]

            def tapview(k, s):
                """(32, 2,16,32) aligned unit-stride view: tap k, all parity-s."""
                b0 = _plane_off(k, s, 0)
                b1 = _plane_off(k, s, 1)
                return _fview(xp_all, b0, [[b1 - b0, 2], [2 * PLP, 16], [1, 32]])

            def tapgrid(k, s, t):
                """(32, 16,32) view: tap k at grid t of parity s."""
                return _fview(xp_all, _plane_off(k, s, t), [[2 * PLP, 16], [1, 32]])

            # --- chains + A tensors (all DVE; bf16 2x mode) ---
            A = {}
            for s in (0, 1):
                c = CHAIN_TAPS[s]
                chain = {1: tapview(c[0], s)}
                for lvl in (2, 3, 4):
                    ct = big.tile([32, 2, 16, 32], BF16, tag=f"ch{s}{lvl}")
                    nc.vector.tensor_mul(ct[:, :, :, :], tapview(c[lvl - 1], s), chain[lvl - 1])
                    chain[lvl] = ct[:, :, :, :]
                for kj in range(3):
                    at = big.tile([96, 2, 16, 32], BF16, tag=f"A{s}{kj}")
                    A[(s, kj)] = at
                    for ki in range(3):
                        k = ki * 3 + kj
                        lvl = SIGMA[s][k]
                        if lvl == 0:
                            continue  # raw tap: direct plane-view matmul
                        nc.vector.tensor_mul(
                            at[32 * ki : 32 * ki + 32, :, :, :],
                            tapview(k, s),
                            chain[lvl],
                        )

            # --- matmuls + epilogue per (parity, grid) ---
            bsn = {0: "b0", 1: "b1"}
            for s in (0, 1):
                for t in (0, 1):
                    ps_conv = pp.tile([64, 16, 32], F32, tag="ps_conv")
                    ps_aeg = pp.tile([64, 16, 32], F32, tag="ps_aeg")
                    # dense conv: 9 K=32 chunks straight off the planes
                    for k in range(9):
                        nc.tensor.matmul(
                            ps_conv[:, :, :],
                            wts["wc"][:, k, :],
                            tapgrid(k, s, t),
                            start=(k == 0),
                            stop=(k == 8),
                        )
                    # aeg: kj-chunks on the materialized A tensors
                    bt = wts[bsn[s]]
                    mms = []
                    for kj in range(3):
                        if kj in RAW[s]:
                            slot = {(0, 1): 0, (0, 2): 1, (1, 2): 2}[(s, kj)]
                            mms.append((bt[0:64, kj, :], A[(s, kj)][0:64, t, :, :]))
                            mms.append((wts["braw"][:, slot, :], tapgrid(RAW[s][kj], s, t)))
                        else:
                            mms.append((bt[:, kj, :], A[(s, kj)][:, t, :, :]))
                    for i, (lh, rh) in enumerate(mms):
                        nc.tensor.matmul(
                            ps_aeg[:, :, :],
                            lh,
                            rh,
                            start=(i == 0),
                            stop=(i == len(mms) - 1),
                        )
                    sig = sigp.tile([64, 16, 32], F32)
                    nc.scalar.activation(
                        sig[:, :, :], ps_aeg[:, :, :],
                        mybir.ActivationFunctionType.Sigmoid,
                    )
                    # absorb the ACT-completion wait so the STT below only
                    # waits on PSUM (one embedded sync wait max)
                    nc.vector.tensor_copy(scratch[0:1, 0:1], sig[0:1, 0:1, 0:1])
                    # out = (conv + bias) * sigmoid(aeg), scattered to parity pixels
                    ov = _fview(out_sb[:, :], 64 * t + (s ^ t), [[128, 16], [2, 32]])
                    nc.vector.scalar_tensor_tensor(
                        out=ov,
                        in0=ps_conv[:, :, :],
                        scalar=bias_t[:, 0:1],
                        in1=sig[:, :, :],
                        op0=mybir.AluOpType.add,
                        op1=mybir.AluOpType.mult,
                    )

            for i in range(8):
                dmae[i % 2].dma_start(
                    out=out_d[8 * i : 8 * i + 8, :],
                    in_=out_sb[8 * i : 8 * i + 8, :],
                )
    nc.finalize()
    return nc


def _host_prep(x, weight, conv_w, conv_b):
    """Shard + pack per-core inputs (bf16 parity planes + weight products)."""
    bf16 = ml_dtypes.bfloat16
    xp = np.pad(np.ascontiguousarray(x, np.float32),
                ((0, 0), (0, 0), (PAD, PAD), (PAD, PAD)))
    kflat = weight.reshape(COUT, CIN, 9).transpose(2, 0, 1)  # (9, cout, cin)
    B = np.zeros((2, 9, COUT, CIN), np.float32)
    for s in (0, 1):
        suf = np.ones((COUT, CIN), np.float32)
        for k in range(8, -1, -1):
            B[s, k] = kflat[k] * suf
            if k % 2 == s:
                suf = suf * kflat[k]
    # conv lhsT: (32, 9, 64): [cin, k, cout]
    wc_p = np.ascontiguousarray(
        conv_w.reshape(COUT, CIN, 9).transpose(1, 2, 0), np.float32
    ).astype(bf16)
    b_p = np.zeros((2, 96, 3, COUT), np.float32)
    for s in (0, 1):
        for kj in range(3):
            for ki in range(3):
                b_p[s, 32 * ki : 32 * ki + 32, kj] = B[s, ki * 3 + kj].T
    b_p = b_p.astype(bf16)
    braw_p = np.zeros((CIN, 3, COUT), np.float32)
    braw_p[:, 0] = B[0, 7].T
    braw_p[:, 1] = B[0, 8].T
    braw_p[:, 2] = B[1, 8].T
    braw_p = braw_p.astype(bf16)
    bias_p = np.ascontiguousarray(conv_b.reshape(COUT, 1), np.float32)

    in_maps = []
    for core in range(N_CORES):
        n, h = divmod(core, 2)
        slab = xp[n, :, 32 * h : 32 * h + ROWS, :]  # (32, 34, 66) f32
        planes = np.zeros((4, CIN, ROWS, PLP), np.float32)
        for q in (0, 1):
            for r in range(ROWS):
                b = (q + r) % 2
                cols = slab[:, r, b::2]  # 33 columns
                planes[q, :, r, : cols.shape[1]] = cols
        planes[2, :, :, :PLP - 1] = planes[0, :, :, 1:]
        planes[3, :, :, :PLP - 1] = planes[1, :, :, 1:]
        xp_core = np.ascontiguousarray(
            planes.transpose(1, 0, 2, 3).reshape(CIN, 4 * PLSZ)
        ).astype(bf16)
        in_maps.append({
            "xp": xp_core,
            "wc": wc_p,
            "b0": b_p[0],
            "b1": b_p[1],
            "braw": braw_p,
            "bias": bias_p,
        })
    return in_maps


_nc_cache = None


def kernel(x, weight, conv_w, conv_b, trace=False):
    global _nc_cache, _last_results
    x = np.asarray(x, np.float32)
    weight = np.asarray(weight, np.float32)
    conv_w = np.asarray(conv_w, np.float32)
    conv_b = np.asarray(conv_b, np.float32)

    if _nc_cache is None:
        _nc_cache = build_nc()
    nc = _nc_cache
    in_maps = _host_prep(x, weight, conv_w, conv_b)
    res = run_bass_kernel_spmd(nc, in_maps, core_ids=list(range(N_CORES)), trace=trace)
    _last_results = res

    out = np.empty((N, COUT, H, W), np.float32)
    for core in range(N_CORES):
        n, h = divmod(core, 2)
        out[n, :, 32 * h : 32 * h + 32, :] = (
            res.results[core]["out"].astype(np.float32).reshape(COUT, OH, OW)
        )
    return out
